# revision 7
# baseline (speedup 1.0000x reference)
"""Causal attention with ALiBi for Trainium2, tensor-parallel over heads x
data-parallel over batch (8 NeuronCores).

Problem: B=4, S=2048, D=2048, NH=16, HD=128, fp32.
  q/k/v = x @ Wq/Wk/Wv ; scores = q k^T / sqrt(HD) + alibi ; causal softmax ;
  out = (probs @ v) @ Wo

Sharding: core (b, j) handles batch b and the 8 interleaved heads
  j, j+2, ..., j+14 (interleaving balances steep/shallow ALiBi slopes so the
  per-core block-skipping is symmetric).  Each core returns out_partial^T;
  the host sums the two per-batch partials and transposes back.

On-core pipeline (bf16 operands on the QK and V paths; PSUM accumulation is
always fp32, the output projection runs in float32r):
  XT = x^T arrives pre-transposed from the host as bf16 [128, 16, 2048],
  streamed in s-slices so head 0 starts projecting at ~6us.
  Per head, a staggered, software-pipelined schedule per s-slice st:
    Q^T[:, st] = Wq-chunks^T @ XT (bf16, fp32 PSUM, DVE copy to SBUF bf16)
    attention blocks for q-tile st whose K/V are already resident (kc < 4st)
    are interleaved with the K^T / V matmuls of this st as PE filler work, so
    the PE never waits on the ACT exp chain (ACT is slower per block than the
    PE's 2 matmuls).  V is computed directly in [k, hd] layout with XT chunks
    as the stationary operand (no PE transposes anywhere).  The 4 diagonal
    blocks run last (their K/V just landed); a gpsimd affine_select masks the
    partial band.  exp blocks also accumulate elementwise on DVE into a
    per-(h,qt) f32r tile; one ones-column matmul per (h,qt) reduces it to the
    softmax sums.  The sums+normalize tail for q-tile st is emitted one stage
    later (during st+1) so the PE never waits on it.
  Softmax details: exp(scores*scale + alibi[k] + shift[q]); the per-q shift
  folds into the ACT bias column per (head, q-tile) for shallow heads (exact
  cancellation) and is applied per-block on DVE for the 2 steep heads.
  Blocks with ALiBi decay < e^-9 of the softmax sum are skipped entirely.
  O^T tiles for qt=0 stay in SBUF; qt>0 spill to DRAM per (h, q-tile).
  out^T = Wo_j^T @ O^T accumulated over the 8 heads (float32r); Wo chunks 0-1
  prefetch during the last head so the output stage starts without a bubble.
"""

import math

import numpy as np

B, S, D, NH = 4, 2048, 2048, 16
HD = D // NH            # 128
NHG = NH // 2           # heads per core
DC = D // 128           # 16 d-chunks
QT_TILES = S // 512     # 4 q tiles
SCALE = 1.0 / math.sqrt(HD)

_cache = {}


def _get_slopes(n):
    def pow2(n):
        start = 2 ** (-(2 ** (-(math.log2(n) - 3))))
        return [start * start**i for i in range(n)]

    if math.log2(n).is_integer():
        return pow2(n)
    c = 2 ** math.floor(math.log2(n))
    return pow2(c) + _get_slopes(2 * c)[0::2][: n - c]


def _build():
    import concourse.bacc as bacc
    import concourse.mybir as mybir
    import concourse.tile as tile
    from concourse.bass import ts

    f32 = mybir.dt.float32
    f32r = mybir.dt.float32r
    bf16 = mybir.dt.bfloat16
    Exp = mybir.ActivationFunctionType.Exp

    nc = bacc.Bacc()
    # x arrives pre-transposed (host-side) as bf16 [D, S]
    x_in = nc.declare_dram_parameter("x", [D, S], bf16, isOutput=False)
    wq_in = nc.declare_dram_parameter("wq", [D, NHG * HD], bf16, isOutput=False)
    wk_in = nc.declare_dram_parameter("wk", [D, NHG * HD], bf16, isOutput=False)
    wv_in = nc.declare_dram_parameter("wv", [D, NHG * HD], bf16, isOutput=False)
    wo_in = nc.declare_dram_parameter("wo", [NHG * HD, D], f32r, isOutput=False)
    # alibi_b[p, ((h*16+kc)*4+qt)] = -slope_h*(S-1-(kc*128+p)) + C[h,qt]
    # C folds the per-q-tile softmax shift for heads with small slope.
    alibi_b_in = nc.declare_dram_parameter(
        "alibi_b", [128, NHG * DC * QT_TILES], f32, isOutput=False)
    # alibi_q[h, q] = +slope_h * (S-1 - q)   (per-query shift)
    alibi_q_in = nc.declare_dram_parameter("alibi_q", [NHG, S], f32,
                                           isOutput=False)
    ones_col_in = nc.declare_dram_parameter("ones_col", [128, 1], f32r,
                                            isOutput=False)
    outT = nc.declare_dram_parameter("outT", [D, S], f32, isOutput=True)

    ot_scratch = nc.dram_tensor("ot_scratch", [NHG, 128, S], f32r)

    # heads are interleaved across the two cores of a batch (core parity j
    # gets global heads j, j+2, ...).  Skip counts use the SHALLOWER
    # parity's slope so one SPMD program is valid for both.
    slope_c = [0.7071067811865476 ** (2 * hh + 2) for hh in range(NHG)]

    def n_skip(h, qt):
        # contribution of a skipped block is < e^-9 of the softmax sum
        dist = int(9.0 / slope_c[h]) + 1
        return max(0, (512 * qt - dist - 127) // 128 + 1)

    with tile.TileContext(nc) as tc:
        with (
            tc.tile_pool(name="consts", bufs=1) as pc,
            tc.tile_pool(name="oz", bufs=8) as po0,
            tc.tile_pool(name="wohi", bufs=1) as pwo_hi,
            tc.tile_pool(name="psA", bufs=2, space="PSUM") as psA,
            tc.tile_pool(name="psST", bufs=4, space="PSUM") as psST,
        ):
            alibi_sb = pc.tile([128, NHG * DC * QT_TILES], f32,
                               name="alibi_sb")
            ones_col = pc.tile([128, 1], f32r, name="ones_col_sb")

            ot0_tiles = {}       # per-head qt=0 O^T tiles, kept in SBUF
            wo_cs = {}
            wo_view = wo_in.rearrange("(h p) f -> p h f", p=128)

            def load_wo(c):
                wo_c = pwo_hi.tile([128, NHG, 512], f32r, tag=f"wo{c}",
                                   name="wo_c")
                nc.sync.dma_start(wo_c[:], wo_view[:, :, ts(c, 512)])
                wo_cs[c] = wo_c

            with (
                tc.tile_pool(name="xt", bufs=1) as pxt,
                tc.tile_pool(name="wp", bufs=2) as pw,
                tc.tile_pool(name="qkv2", bufs=2) as pq2,
                tc.tile_pool(name="qkv", bufs=2) as pq,
                tc.tile_pool(name="att", bufs=2) as pa,
                tc.tile_pool(name="epool", bufs=6) as pe_pool,
                tc.tile_pool(name="small", bufs=2) as psm,
            ):
                XT = pxt.tile([128, DC, S], bf16, name="XT")
                xt_view = x_in.rearrange("(dc p) s -> p dc s", p=128)

                # pending softmax-sum/normalize work, emitted one stage late:
                # (h, qt, pot, eacc)
                pending = []

                def emit_norm():
                    if not pending:
                        return
                    h, qt, pot, eacc = pending.pop()
                    psums = psA.tile([1, 512], f32, tag="pot", name="psums")
                    nc.tensor.matmul(psums[:], ones_col[:], eacc[:],
                                     start=True, stop=True)
                    recip = psm.tile([1, 512], f32, tag="recip", name="recip")
                    nc.vector.reciprocal(recip[:], psums[:])
                    bc_sb = pa.tile([128, 512], f32, tag="bc", name="bc_sb")
                    nc.gpsimd.partition_broadcast(bc_sb[:], recip[:])
                    if qt == 0:
                        ot_sb = po0.tile([128, 512], f32r, tag="ot0",
                                         name="ot0_sb")
                        ot0_tiles[h] = ot_sb
                    else:
                        ot_sb = pa.tile([128, 512], f32r, tag="ot",
                                        name="ot_sb")
                    nc.vector.tensor_mul(out=ot_sb[:], in0=pot[:],
                                         in1=bc_sb[:])
                    if qt != 0:
                        nc.sync.dma_start(ot_scratch[h, :, ts(qt, 512)],
                                          ot_sb[:])

                def emit_head(h):
                    qt_sb = pq2.tile([128, S], bf16, tag="QT", name="qt_sb")
                    kt_sb = pq2.tile([128, S], bf16, tag="KT", name="kt_sb")
                    v_sb = pq.tile([128, DC, HD], bf16, tag="V", name="v_sb")
                    w_sbs = []
                    for w_in, wtag in ((wq_in, "wq"), (wk_in, "wk"),
                                       (wv_in, "wv")):
                        w_sb = pw.tile([128, DC, HD], bf16, tag=wtag,
                                       name="w_sb")
                        if h == NHG - 1 and wtag == "wq":
                            # wq first, then the first x slice (split in two
                            # halves so the PE can start on the first half)
                            nc.sync.dma_start(
                                w_sb[:],
                                w_in[:, ts(h, HD)].rearrange(
                                    "(dc p) f -> p dc f", p=128))
                            for dh in range(2):
                                nc.sync.dma_start(
                                    XT[:, ts(dh, 8), ts(0, 512)],
                                    xt_view[:, ts(dh, 8), ts(0, 512)])
                            nc.sync.dma_start(alibi_sb[:], alibi_b_in[:])
                            nc.sync.dma_start(ones_col[:], ones_col_in[:])
                        else:
                            nc.sync.dma_start(
                                w_sb[:],
                                w_in[:, ts(h, HD)].rearrange(
                                    "(dc p) f -> p dc f", p=128))
                        w_sbs.append(w_sb)
                    if h == NHG - 1:
                        for st in range(1, QT_TILES):
                            nc.sync.dma_start(XT[:, :, ts(st, 512)],
                                              xt_view[:, :, ts(st, 512)])
                    if h == 0:
                        load_wo(0)
                        load_wo(1)

                    steep = h < 2
                    for st in range(QT_TILES):
                        qt = st
                        nkc = 4 * (qt + 1)
                        kc0 = n_skip(h, qt)
                        if steep:
                            shift_sb = psm.tile([128, 512], f32, tag="shift",
                                                name="shift_sb")
                            nc.sync.dma_start(
                                shift_sb[:],
                                alibi_q_in[h, ts(qt, 512)]
                                .partition_broadcast(128))

                        # ---- Q projection for this s-slice ----
                        pp_q = psA.tile([128, 512], f32, tag="pp", name="pp_q")
                        for dc in range(DC):
                            nc.tensor.matmul(
                                pp_q[:], w_sbs[0][:, dc, :],
                                XT[:, dc, ts(st, 512)],
                                start=(dc == 0), stop=(dc == DC - 1),
                                skip_group_check=True)
                        nc.vector.tensor_copy(qt_sb[:, ts(st, 512)], pp_q[:])

                        # softmax sums + normalize of the PREVIOUS q-tile
                        emit_norm()

                        # ---- filler stream: K^T and V matmuls of this
                        # s-slice, interleaved between attention blocks so
                        # the PE never drains while ACT runs the exp chain.
                        pp_k = psA.tile([128, 512], f32, tag="pp", name="pp_k")
                        pp_v = psA.tile([128, 512], f32, tag="pp", name="pp_v")

                        def fill_units():
                            for dc in range(DC):
                                yield ("K", dc)
                            for j in range(4):
                                for dc in range(DC):
                                    yield ("V", j, dc)

                        filler = fill_units()

                        def take_fillers(n):
                            for _ in range(n):
                                u = next(filler, None)
                                if u is None:
                                    return
                                if u[0] == "K":
                                    dc = u[1]
                                    nc.tensor.matmul(
                                        pp_k[:], w_sbs[1][:, dc, :],
                                        XT[:, dc, ts(st, 512)],
                                        start=(dc == 0), stop=(dc == DC - 1),
                                        skip_group_check=True)
                                    if dc == DC - 1:
                                        nc.vector.tensor_copy(
                                            kt_sb[:, ts(st, 512)], pp_k[:])
                                else:
                                    _, j, dc = u
                                    sc = st * 4 + j
                                    nc.tensor.matmul(
                                        pp_v[:, ts(j, 128)],
                                        XT[:, dc, ts(sc, 128)],
                                        w_sbs[2][:, dc, :],
                                        start=(dc == 0), stop=(dc == DC - 1),
                                        skip_group_check=True)
                                    if dc == DC - 1:
                                        nc.vector.tensor_copy(
                                            v_sb[:, sc, :],
                                            pp_v[:, ts(j, 128)])

                        pot = psA.tile([128, 512], f32, tag="pot", name="pot")
                        eacc = pa.tile([128, 512], f32r, tag="eacc",
                                       name="eacc")

                        def attn_block(kc):
                            # diag blocks: columns < r are fully masked and
                            # not computed (bf16 matmuls run 1 cyc/row at
                            # any width)
                            r = max(0, 128 * kc - 512 * qt)
                            pst = psST.tile([128, 512], f32, tag="pst",
                                            name="pst")
                            nc.tensor.matmul(pst[:, r:],
                                             kt_sb[:, ts(kc, 128)],
                                             qt_sb[:, 512 * qt + r:
                                                   512 * (qt + 1)],
                                             start=True, stop=True)
                            e_sb = pe_pool.tile([128, 512], bf16, tag="e",
                                                name="e_sb")
                            col = (h * DC + kc) * QT_TILES + qt
                            if steep:
                                t1 = pa.tile([128, 512], f32, tag="t1",
                                             name="t1")
                                nc.vector.scalar_tensor_tensor(
                                    t1[:, r:], pst[:, r:], SCALE,
                                    shift_sb[:, r:],
                                    mybir.AluOpType.mult,
                                    mybir.AluOpType.add)
                                nc.scalar.activation(
                                    e_sb[:, r:], t1[:, r:], Exp,
                                    bias=alibi_sb[:, col:col + 1],
                                    scale=1.0)
                            else:
                                nc.scalar.activation(
                                    e_sb[:, r:], pst[:, r:], Exp,
                                    bias=alibi_sb[:, col:col + 1],
                                    scale=SCALE)
                            if kc >= 4 * qt:
                                # keep where qf - kp - r >= 0 (k <= q)
                                nc.gpsimd.affine_select(
                                    e_sb[:, r:r + 128],
                                    e_sb[:, r:r + 128],
                                    pattern=[[1, 128]],
                                    compare_op=mybir.AluOpType.is_ge,
                                    fill=0.0,
                                    base=0,
                                    channel_multiplier=-1)
                            # accumulate exp blocks for the softmax sums
                            if kc == kc0:
                                nc.vector.tensor_copy(eacc[:, r:],
                                                      e_sb[:, r:])
                            else:
                                nc.vector.tensor_add(eacc[:, r:],
                                                     eacc[:, r:],
                                                     e_sb[:, r:])
                            nc.tensor.matmul(pot[:, r:], v_sb[:, kc, :],
                                             e_sb[:, r:],
                                             start=(kc == kc0),
                                             stop=(kc == nkc - 1))

                        take_fillers(2)
                        # blocks whose K/V are resident from earlier s-slices
                        for kc in range(kc0, 4 * qt):
                            attn_block(kc)
                            take_fillers(2)
                        take_fillers(DC + 4 * DC)   # drain the rest
                        # diagonal blocks (K/V of this s-slice just landed)
                        for kc in range(4 * qt, nkc):
                            attn_block(kc)
                        pending.append((h, qt, pot, eacc))

                for h in range(NHG - 1, -1, -1):
                    emit_head(h)
                emit_norm()

            # ---- out^T = Wo_g^T @ O^T (XT pool closed) ----
            with (
                tc.tile_pool(name="wo", bufs=1) as pwo,
                tc.tile_pool(name="otl", bufs=2) as pot_l,
                tc.tile_pool(name="ost", bufs=2) as post,
            ):
                def load_wo_lo(c):
                    wo_c = pwo.tile([128, NHG, 512], f32r, tag=f"wo{c}",
                                    name="wo_c")
                    nc.gpsimd.dma_start(wo_c[:], wo_view[:, :, ts(c, 512)])
                    wo_cs[c] = wo_c

                for st in range(QT_TILES):
                    if st == 0:
                        ot_of = lambda h: ot0_tiles[h][:]
                    else:
                        ot_all = pot_l.tile([128, NHG, 512], f32r,
                                            tag="ot_all", name="ot_all")
                        for h in range(NHG):
                            nc.sync.dma_start(ot_all[:, h, :],
                                              ot_scratch[h, :, ts(st, 512)])
                        ot_of = lambda h, _t=ot_all: _t[:, h, :]
                    for mt in range(D // 128):
                        # stream in the second half of Wo behind the first
                        # matmul groups (a big DMA emitted before dependent
                        # PE work would stall it on the DGE ring semaphore)
                        if st == 0 and mt == 0:
                            load_wo_lo(2)
                            load_wo_lo(3)
                        pp = psA.tile([128, 512], f32, tag="pp", name="pp")
                        for h in range(NHG):
                            nc.tensor.matmul(
                                pp[:],
                                wo_cs[mt // 4][:, h, ts(mt % 4, 128)],
                                ot_of(h),
                                start=(h == 0), stop=(h == NHG - 1))
                        o_sb = post.tile([128, 512], f32, tag="osb",
                                         name="o_sb")
                        nc.scalar.copy(o_sb[:], pp[:])
                        nc.sync.dma_start(outT[ts(mt, 128), ts(st, 512)],
                                          o_sb[:])

    nc.compile()
    return nc


def _in_maps(x, Wq, Wk, Wv, Wo):
    import ml_dtypes

    bf16 = ml_dtypes.bfloat16
    slopes = np.asarray(_get_slopes(NH), dtype=np.float32)
    pos = np.arange(S, dtype=np.float32)
    dist = np.float32(S - 1) - pos                       # (S,)
    ones_col = np.ones((128, 1), np.float32)

    in_maps = []
    for b in range(B):
        xT = np.ascontiguousarray(x[b].T).astype(bf16)
        for g in range(2):
            heads = list(range(g, NH, 2))                 # interleaved
            sl = slopes[heads]                            # (8,)
            # alibi_b[p, ((h*DC+kc)*QT+qt)] = -sl[h]*dist[kc*128+p] + C[h,qt]
            ab = np.empty((128, NHG * DC * QT_TILES), np.float32)
            d2 = dist.reshape(DC, 128)                    # [kc, p]
            for h in range(NHG):
                for kc in range(DC):
                    a_col = (-sl[h] * d2[kc]).astype(np.float32)  # (128,)
                    for qt in range(QT_TILES):
                        if h < 2:
                            c = np.float32(0.0)
                        else:
                            q_mid = 512 * qt + 255.5
                            c = np.float32(sl[h] * (S - 1 - q_mid))
                        ab[:, (h * DC + kc) * QT_TILES + qt] = a_col + c
            alibi_q = (sl[:, None] * dist[None, :]).astype(np.float32)
            in_maps.append({
                "x": xT,
                "wq": np.concatenate(
                    [Wq[:, h * HD:(h + 1) * HD] for h in heads],
                    axis=1).astype(bf16),
                "wk": np.concatenate(
                    [Wk[:, h * HD:(h + 1) * HD] for h in heads],
                    axis=1).astype(bf16),
                "wv": np.concatenate(
                    [Wv[:, h * HD:(h + 1) * HD] for h in heads],
                    axis=1).astype(bf16),
                "wo": np.concatenate(
                    [Wo[h * HD:(h + 1) * HD, :] for h in heads], axis=0),
                "alibi_b": ab,
                "alibi_q": alibi_q,
                "ones_col": ones_col,
            })
    return in_maps


def kernel(x, Wq, Wk, Wv, Wo, _trace=False):
    from concourse.bass_utils import run_bass_kernel_spmd

    if "nc" not in _cache:
        _cache["nc"] = _build()
    nc = _cache["nc"]

    res = run_bass_kernel_spmd(
        nc, _in_maps(x, Wq, Wk, Wv, Wo), core_ids=list(range(2 * B)),
        trace=_trace)
    _cache["last_exec_time_ns"] = res.exec_time_ns

    out = np.empty((B, S, D), dtype=np.float32)
    for b in range(B):
        out[b] = (res.results[2 * b]["outT"] + res.results[2 * b + 1]["outT"]).T
    return out


# revision 8
# speedup vs baseline: 1.0308x; 1.0308x over previous
"""Causal attention with ALiBi for Trainium2, tensor-parallel over heads x
data-parallel over batch (8 NeuronCores).

Problem: B=4, S=2048, D=2048, NH=16, HD=128, fp32.
  q/k/v = x @ Wq/Wk/Wv ; scores = q k^T / sqrt(HD) + alibi ; causal softmax ;
  out = (probs @ v) @ Wo

Sharding: core (b, j) handles batch b and the 8 interleaved heads
  j, j+2, ..., j+14 (interleaving balances steep/shallow ALiBi slopes so the
  per-core block-skipping is symmetric).  Each core returns out_partial^T;
  the host sums the two per-batch partials and transposes back.

On-core pipeline (bf16 operands on the QK and V paths; PSUM accumulation is
always fp32, the output projection runs in float32r):
  XT = x^T arrives pre-transposed from the host as bf16 [128, 16, 2048],
  streamed in s-slices so head 0 starts projecting at ~6us.
  Per head, a staggered, software-pipelined schedule per s-slice st:
    Q^T[:, st] = Wq-chunks^T @ XT (bf16, fp32 PSUM, DVE copy to SBUF bf16)
    attention blocks for q-tile st whose K/V are already resident (kc < 4st)
    are interleaved with the K^T / V matmuls of this st as PE filler work, so
    the PE never waits on the ACT exp chain (ACT is slower per block than the
    PE's 2 matmuls).  V is computed directly in [k, hd] layout with XT chunks
    as the stationary operand (no PE transposes anywhere).  The 4 diagonal
    blocks run last (their K/V just landed); a gpsimd affine_select masks the
    partial band.  exp blocks also accumulate elementwise on DVE into a
    per-(h,qt) f32r tile; one ones-column matmul per (h,qt) reduces it to the
    softmax sums.  The sums+normalize tail for q-tile st is emitted one stage
    later (during st+1) so the PE never waits on it.
  Softmax details: exp(scores*scale + alibi[k] + shift[q]); the per-q shift
  folds into the ACT bias column per (head, q-tile) for shallow heads (exact
  cancellation) and is applied per-block on DVE for the 2 steep heads.
  Blocks with ALiBi decay < e^-9 of the softmax sum are skipped entirely.
  O^T tiles for qt=0 stay in SBUF; qt>0 spill to DRAM per (h, q-tile).
  out^T = Wo_j^T @ O^T accumulated over the 8 heads (float32r); Wo chunks 0-1
  prefetch during the last head so the output stage starts without a bubble.
"""

import math

import numpy as np

B, S, D, NH = 4, 2048, 2048, 16
HD = D // NH            # 128
NHG = NH // 2           # heads per core
DC = D // 128           # 16 d-chunks
QT_TILES = S // 512     # 4 q tiles
SCALE = 1.0 / math.sqrt(HD)

_cache = {}


def _get_slopes(n):
    def pow2(n):
        start = 2 ** (-(2 ** (-(math.log2(n) - 3))))
        return [start * start**i for i in range(n)]

    if math.log2(n).is_integer():
        return pow2(n)
    c = 2 ** math.floor(math.log2(n))
    return pow2(c) + _get_slopes(2 * c)[0::2][: n - c]


def _build():
    import concourse.bacc as bacc
    import concourse.mybir as mybir
    import concourse.tile as tile
    from concourse.bass import ts

    f32 = mybir.dt.float32
    f32r = mybir.dt.float32r
    bf16 = mybir.dt.bfloat16
    Exp = mybir.ActivationFunctionType.Exp

    nc = bacc.Bacc()
    # x arrives pre-transposed (host-side) as bf16 [D, S]
    x_in = nc.declare_dram_parameter("x", [D, S], bf16, isOutput=False)
    wq_in = nc.declare_dram_parameter("wq", [D, NHG * HD], bf16, isOutput=False)
    wk_in = nc.declare_dram_parameter("wk", [D, NHG * HD], bf16, isOutput=False)
    wv_in = nc.declare_dram_parameter("wv", [D, NHG * HD], bf16, isOutput=False)
    wo_in = nc.declare_dram_parameter("wo", [NHG * HD, D], bf16, isOutput=False)
    # alibi_b[p, ((h*16+kc)*4+qt)] = -slope_h*(S-1-(kc*128+p)) + C[h,qt]
    # C folds the per-q-tile softmax shift for heads with small slope.
    alibi_b_in = nc.declare_dram_parameter(
        "alibi_b", [128, NHG * DC * QT_TILES], f32, isOutput=False)
    # alibi_q[h, q] = +slope_h * (S-1 - q)   (per-query shift)
    alibi_q_in = nc.declare_dram_parameter("alibi_q", [NHG, S], f32,
                                           isOutput=False)
    ones_col_in = nc.declare_dram_parameter("ones_col", [128, 1], f32r,
                                            isOutput=False)
    outT = nc.declare_dram_parameter("outT", [D, S], f32, isOutput=True)

    ot_scratch = nc.dram_tensor("ot_scratch", [NHG, 128, S], bf16)

    # heads are interleaved across the two cores of a batch (core parity j
    # gets global heads j, j+2, ...).  Skip counts use the SHALLOWER
    # parity's slope so one SPMD program is valid for both.
    slope_c = [0.7071067811865476 ** (2 * hh + 2) for hh in range(NHG)]

    def n_skip(h, qt):
        # contribution of a skipped block is < e^-9 of the softmax sum
        dist = int(9.0 / slope_c[h]) + 1
        return max(0, (512 * qt - dist - 127) // 128 + 1)

    with tile.TileContext(nc) as tc:
        with (
            tc.tile_pool(name="consts", bufs=1) as pc,
            tc.tile_pool(name="oz", bufs=8) as po0,
            tc.tile_pool(name="wohi", bufs=1) as pwo_hi,
            tc.tile_pool(name="psA", bufs=2, space="PSUM") as psA,
            tc.tile_pool(name="psST", bufs=4, space="PSUM") as psST,
        ):
            alibi_sb = pc.tile([128, NHG * DC * QT_TILES], f32,
                               name="alibi_sb")
            ones_col = pc.tile([128, 1], f32r, name="ones_col_sb")

            ot0_tiles = {}       # per-head qt=0 O^T tiles, kept in SBUF
            wo_cs = {}
            wo_view = wo_in.rearrange("(h p) f -> p h f", p=128)

            def load_wo(c):
                wo_c = pwo_hi.tile([128, NHG, 512], bf16, tag=f"wo{c}",
                                   name="wo_c")
                nc.sync.dma_start(wo_c[:], wo_view[:, :, ts(c, 512)])
                wo_cs[c] = wo_c

            with (
                tc.tile_pool(name="xt", bufs=1) as pxt,
                tc.tile_pool(name="wp", bufs=2) as pw,
                tc.tile_pool(name="qkv2", bufs=2) as pq2,
                tc.tile_pool(name="qkv", bufs=2) as pq,
                tc.tile_pool(name="att", bufs=2) as pa,
                tc.tile_pool(name="epool", bufs=6) as pe_pool,
                tc.tile_pool(name="small", bufs=2) as psm,
            ):
                XT = pxt.tile([128, DC, S], bf16, name="XT")
                xt_view = x_in.rearrange("(dc p) s -> p dc s", p=128)

                # pending softmax-sum/normalize work, emitted one stage late:
                # (h, qt, pot, eacc)
                pending = []

                def emit_norm():
                    if not pending:
                        return
                    h, qt, pot, eacc = pending.pop()
                    psums = psA.tile([1, 512], f32, tag="pot", name="psums")
                    nc.tensor.matmul(psums[:], ones_col[:], eacc[:],
                                     start=True, stop=True)
                    recip = psm.tile([1, 512], f32, tag="recip", name="recip")
                    nc.vector.reciprocal(recip[:], psums[:])
                    bc_sb = pa.tile([128, 512], f32, tag="bc", name="bc_sb")
                    nc.gpsimd.partition_broadcast(bc_sb[:], recip[:])
                    if qt == 0:
                        ot_sb = po0.tile([128, 512], bf16, tag="ot0",
                                         name="ot0_sb")
                        ot0_tiles[h] = ot_sb
                    else:
                        ot_sb = pa.tile([128, 512], bf16, tag="ot",
                                        name="ot_sb")
                    nc.vector.tensor_mul(out=ot_sb[:], in0=pot[:],
                                         in1=bc_sb[:])
                    if qt != 0:
                        nc.sync.dma_start(ot_scratch[h, :, ts(qt, 512)],
                                          ot_sb[:])

                def emit_head(h):
                    qt_sb = pq2.tile([128, S], bf16, tag="QT", name="qt_sb")
                    kt_sb = pq2.tile([128, S], bf16, tag="KT", name="kt_sb")
                    v_sb = pq.tile([128, DC, HD], bf16, tag="V", name="v_sb")
                    w_sbs = []
                    for w_in, wtag in ((wq_in, "wq"), (wk_in, "wk"),
                                       (wv_in, "wv")):
                        w_sb = pw.tile([128, DC, HD], bf16, tag=wtag,
                                       name="w_sb")
                        if h == NHG - 1 and wtag == "wq":
                            # wq first, then the first x slice (split in two
                            # halves so the PE can start on the first half)
                            nc.sync.dma_start(
                                w_sb[:],
                                w_in[:, ts(h, HD)].rearrange(
                                    "(dc p) f -> p dc f", p=128))
                            for dh in range(2):
                                nc.sync.dma_start(
                                    XT[:, ts(dh, 8), ts(0, 512)],
                                    xt_view[:, ts(dh, 8), ts(0, 512)])
                            nc.sync.dma_start(alibi_sb[:], alibi_b_in[:])
                            nc.sync.dma_start(ones_col[:], ones_col_in[:])
                        else:
                            nc.sync.dma_start(
                                w_sb[:],
                                w_in[:, ts(h, HD)].rearrange(
                                    "(dc p) f -> p dc f", p=128))
                        w_sbs.append(w_sb)
                    if h == NHG - 1:
                        for st in range(1, QT_TILES):
                            nc.sync.dma_start(XT[:, :, ts(st, 512)],
                                              xt_view[:, :, ts(st, 512)])
                    if h == 0:
                        for c in range(4):
                            load_wo(c)

                    steep = h < 2
                    for st in range(QT_TILES):
                        qt = st
                        nkc = 4 * (qt + 1)
                        kc0 = n_skip(h, qt)
                        if steep:
                            shift_sb = psm.tile([128, 512], f32, tag="shift",
                                                name="shift_sb")
                            nc.sync.dma_start(
                                shift_sb[:],
                                alibi_q_in[h, ts(qt, 512)]
                                .partition_broadcast(128))

                        # ---- Q projection for this s-slice ----
                        pp_q = psA.tile([128, 512], f32, tag="pp", name="pp_q")
                        for dc in range(DC):
                            nc.tensor.matmul(
                                pp_q[:], w_sbs[0][:, dc, :],
                                XT[:, dc, ts(st, 512)],
                                start=(dc == 0), stop=(dc == DC - 1),
                                skip_group_check=True)
                        nc.vector.tensor_copy(qt_sb[:, ts(st, 512)], pp_q[:])

                        # softmax sums + normalize of the PREVIOUS q-tile
                        emit_norm()

                        # ---- filler stream: K^T and V matmuls of this
                        # s-slice, interleaved between attention blocks so
                        # the PE never drains while ACT runs the exp chain.
                        pp_k = psA.tile([128, 512], f32, tag="pp", name="pp_k")
                        pp_v = psA.tile([128, 512], f32, tag="pp", name="pp_v")

                        def fill_units():
                            for dc in range(DC):
                                yield ("K", dc)
                            for j in range(4):
                                for dc in range(DC):
                                    yield ("V", j, dc)

                        filler = fill_units()

                        def take_fillers(n):
                            for _ in range(n):
                                u = next(filler, None)
                                if u is None:
                                    return
                                if u[0] == "K":
                                    dc = u[1]
                                    nc.tensor.matmul(
                                        pp_k[:], w_sbs[1][:, dc, :],
                                        XT[:, dc, ts(st, 512)],
                                        start=(dc == 0), stop=(dc == DC - 1),
                                        skip_group_check=True)
                                    if dc == DC - 1:
                                        nc.vector.tensor_copy(
                                            kt_sb[:, ts(st, 512)], pp_k[:])
                                else:
                                    _, j, dc = u
                                    sc = st * 4 + j
                                    nc.tensor.matmul(
                                        pp_v[:, ts(j, 128)],
                                        XT[:, dc, ts(sc, 128)],
                                        w_sbs[2][:, dc, :],
                                        start=(dc == 0), stop=(dc == DC - 1),
                                        skip_group_check=True)
                                    if dc == DC - 1:
                                        nc.vector.tensor_copy(
                                            v_sb[:, sc, :],
                                            pp_v[:, ts(j, 128)])

                        pot = psA.tile([128, 512], f32, tag="pot", name="pot")
                        eacc = pa.tile([128, 512], f32r, tag="eacc",
                                       name="eacc")

                        def attn_block(kc):
                            # diag blocks: columns < r are fully masked and
                            # not computed (bf16 matmuls run 1 cyc/row at
                            # any width)
                            r = max(0, 128 * kc - 512 * qt)
                            pst = psST.tile([128, 512], f32, tag="pst",
                                            name="pst")
                            nc.tensor.matmul(pst[:, r:],
                                             kt_sb[:, ts(kc, 128)],
                                             qt_sb[:, 512 * qt + r:
                                                   512 * (qt + 1)],
                                             start=True, stop=True)
                            e_sb = pe_pool.tile([128, 512], bf16, tag="e",
                                                name="e_sb")
                            col = (h * DC + kc) * QT_TILES + qt
                            if steep:
                                t1 = pa.tile([128, 512], f32, tag="t1",
                                             name="t1")
                                nc.vector.scalar_tensor_tensor(
                                    t1[:, r:], pst[:, r:], SCALE,
                                    shift_sb[:, r:],
                                    mybir.AluOpType.mult,
                                    mybir.AluOpType.add)
                                nc.scalar.activation(
                                    e_sb[:, r:], t1[:, r:], Exp,
                                    bias=alibi_sb[:, col:col + 1],
                                    scale=1.0)
                            else:
                                nc.scalar.activation(
                                    e_sb[:, r:], pst[:, r:], Exp,
                                    bias=alibi_sb[:, col:col + 1],
                                    scale=SCALE)
                            if kc >= 4 * qt:
                                # keep where qf - kp - r >= 0 (k <= q)
                                nc.gpsimd.affine_select(
                                    e_sb[:, r:r + 128],
                                    e_sb[:, r:r + 128],
                                    pattern=[[1, 128]],
                                    compare_op=mybir.AluOpType.is_ge,
                                    fill=0.0,
                                    base=0,
                                    channel_multiplier=-1)
                            # accumulate exp blocks for the softmax sums
                            if kc == kc0:
                                nc.vector.tensor_copy(eacc[:, r:],
                                                      e_sb[:, r:])
                            else:
                                nc.vector.tensor_add(eacc[:, r:],
                                                     eacc[:, r:],
                                                     e_sb[:, r:])
                            nc.tensor.matmul(pot[:, r:], v_sb[:, kc, :],
                                             e_sb[:, r:],
                                             start=(kc == kc0),
                                             stop=(kc == nkc - 1))

                        take_fillers(2)
                        # blocks whose K/V are resident from earlier s-slices
                        for kc in range(kc0, 4 * qt):
                            attn_block(kc)
                            take_fillers(2)
                        take_fillers(DC + 4 * DC)   # drain the rest
                        # diagonal blocks (K/V of this s-slice just landed)
                        for kc in range(4 * qt, nkc):
                            attn_block(kc)
                        pending.append((h, qt, pot, eacc))

                for h in range(NHG - 1, -1, -1):
                    emit_head(h)
                emit_norm()

            # ---- out^T = Wo_g^T @ O^T (XT pool closed) ----
            with (
                tc.tile_pool(name="otl", bufs=2) as pot_l,
                tc.tile_pool(name="ost", bufs=2) as post,
            ):
                for st in range(QT_TILES):
                    if st == 0:
                        ot_of = lambda h: ot0_tiles[h][:]
                    else:
                        ot_all = pot_l.tile([128, NHG, 512], bf16,
                                            tag="ot_all", name="ot_all")
                        for h in range(NHG):
                            nc.sync.dma_start(ot_all[:, h, :],
                                              ot_scratch[h, :, ts(st, 512)])
                        ot_of = lambda h, _t=ot_all: _t[:, h, :]
                    for mt in range(D // 128):
                        pp = psA.tile([128, 512], f32, tag="pp", name="pp")
                        for h in range(NHG):
                            nc.tensor.matmul(
                                pp[:],
                                wo_cs[mt // 4][:, h, ts(mt % 4, 128)],
                                ot_of(h),
                                start=(h == 0), stop=(h == NHG - 1))
                        o_sb = post.tile([128, 512], f32, tag="osb",
                                         name="o_sb")
                        nc.scalar.copy(o_sb[:], pp[:])
                        nc.sync.dma_start(outT[ts(mt, 128), ts(st, 512)],
                                          o_sb[:])

    nc.compile()
    return nc


def _in_maps(x, Wq, Wk, Wv, Wo):
    import ml_dtypes

    bf16 = ml_dtypes.bfloat16
    slopes = np.asarray(_get_slopes(NH), dtype=np.float32)
    pos = np.arange(S, dtype=np.float32)
    dist = np.float32(S - 1) - pos                       # (S,)
    ones_col = np.ones((128, 1), np.float32)

    in_maps = []
    for b in range(B):
        xT = np.ascontiguousarray(x[b].T).astype(bf16)
        for g in range(2):
            heads = list(range(g, NH, 2))                 # interleaved
            sl = slopes[heads]                            # (8,)
            # alibi_b[p, ((h*DC+kc)*QT+qt)] = -sl[h]*dist[kc*128+p] + C[h,qt]
            ab = np.empty((128, NHG * DC * QT_TILES), np.float32)
            d2 = dist.reshape(DC, 128)                    # [kc, p]
            for h in range(NHG):
                for kc in range(DC):
                    a_col = (-sl[h] * d2[kc]).astype(np.float32)  # (128,)
                    for qt in range(QT_TILES):
                        if h < 2:
                            c = np.float32(0.0)
                        else:
                            q_mid = 512 * qt + 255.5
                            c = np.float32(sl[h] * (S - 1 - q_mid))
                        ab[:, (h * DC + kc) * QT_TILES + qt] = a_col + c
            alibi_q = (sl[:, None] * dist[None, :]).astype(np.float32)
            in_maps.append({
                "x": xT,
                "wq": np.concatenate(
                    [Wq[:, h * HD:(h + 1) * HD] for h in heads],
                    axis=1).astype(bf16),
                "wk": np.concatenate(
                    [Wk[:, h * HD:(h + 1) * HD] for h in heads],
                    axis=1).astype(bf16),
                "wv": np.concatenate(
                    [Wv[:, h * HD:(h + 1) * HD] for h in heads],
                    axis=1).astype(bf16),
                "wo": np.concatenate(
                    [Wo[h * HD:(h + 1) * HD, :] for h in heads],
                    axis=0).astype(bf16),
                "alibi_b": ab,
                "alibi_q": alibi_q,
                "ones_col": ones_col,
            })
    return in_maps


def kernel(x, Wq, Wk, Wv, Wo, _trace=False):
    from concourse.bass_utils import run_bass_kernel_spmd

    if "nc" not in _cache:
        _cache["nc"] = _build()
    nc = _cache["nc"]

    res = run_bass_kernel_spmd(
        nc, _in_maps(x, Wq, Wk, Wv, Wo), core_ids=list(range(2 * B)),
        trace=_trace)
    _cache["last_exec_time_ns"] = res.exec_time_ns

    out = np.empty((B, S, D), dtype=np.float32)
    for b in range(B):
        out[b] = (res.results[2 * b]["outT"] + res.results[2 * b + 1]["outT"]).T
    return out
